# revision 4
# baseline (speedup 1.0000x reference)
"""LGConv (GNN message passing) on 8 Trainium2 NeuronCores — v3.

emb = segment_sum(edge_weight * src_x[src_idx], dst_idx, 100000)

Instruction-count-minimal design (dispatch costs ~90us/instruction here):
  - dst nodes sharded across 8 cores (12500 each), no collectives.
  - Per core: big DMAGather calls pull src rows; one DVE tensor_tensor per
    call scales by edge weight (stride-0 broadcast along features);
    DMAScatterAdd CCE-adds message rows into a K-plane DRAM buffer
    wide[k*12544 + dst]. Plane k = (occurrence of dst within its src-quarter)
    % K with K >= max per-quarter degree, so no two scatter descriptors ever
    target the same address — no RMW ordering hazards at all.
  - Two planes share one scatter call via idx = (k%2)*12544 + dst (< 2^15).
  - wide is a DRAM tile zeroed by dma_start before the scatters (tile deps
    order zero -> scatter -> reduce); a final DVE tensor_reduce over the K
    planes produces emb.
  - Pad positions gather row 0 with weight 0 and scatter into scratch rows
    12500..12543 of plane 0 (never read back; first 12500 rows returned).
"""

import numpy as np

N_NODES = 100000
N_EDGES = 1250000
D = 64
NCORES = 8
NPC = N_NODES // NCORES  # 12500
NQ = 4
SRC_CHUNK = N_NODES // NQ  # 25000
CAP = 8064  # idxs per gather/scatter call (Q7 scratch + desc-ring safe)
ROWS = 12544  # 12500 real dst rows + scratch, multiple of 128


def _prep(edge_index, edge_weight):
    src = np.asarray(edge_index[0], dtype=np.int64)
    dst = np.asarray(edge_index[1], dtype=np.int64)
    w = np.asarray(edge_weight, dtype=np.float32).reshape(-1)
    E = src.size

    core = dst // NPC
    dloc = dst - core * NPC
    q = src // SRC_CHUNK
    sloc = (src - q * SRC_CHUNK).astype(np.int16)

    # occurrence rank of each edge within its (quarter, dst) group
    key_qd = q * N_NODES + dst
    o = np.argsort(key_qd, kind="stable")
    ks = key_qd[o]
    run_start = np.empty(E, bool)
    run_start[0] = True
    run_start[1:] = ks[1:] != ks[:-1]
    run_id = np.cumsum(run_start) - 1
    starts = np.flatnonzero(run_start)
    occ = np.empty(E, np.int64)
    occ[o] = np.arange(E) - starts[run_id]

    K = max(16, int(-(-int(occ.max() + 1) // 8)) * 8)  # planes; dup-free
    npair = K // 2
    k = occ % K
    pair = k // 2
    kl = k % 2

    seg = (core * NQ + q) * npair + pair
    order = np.argsort(seg, kind="stable")
    ss = seg[order]
    sstart = np.empty(E, bool)
    sstart[0] = True
    sstart[1:] = ss[1:] != ss[:-1]
    sid = np.cumsum(sstart) - 1
    sstarts = np.flatnonzero(sstart)
    rank = np.arange(E) - sstarts[sid]

    counts = np.bincount(seg, minlength=NCORES * NQ * npair)
    counts = counts.reshape(NCORES, NQ * npair)
    segchunks = (-(-counts // 128)).max(axis=0)  # [NQ*npair] shared over cores
    seg_base = np.concatenate([[0], np.cumsum(segchunks * 128)])
    epad = int(seg_base[-1])

    lseg = (q * npair + pair)  # core-local segment id, shared layout
    pos = seg_base[lseg[order]] + rank
    core_o = core[order]

    gidx = np.zeros((NCORES, epad), np.int16)
    pads = (12500 + (np.arange(epad) % 44)).astype(np.int16)
    sidx = np.repeat(pads[None, :], NCORES, axis=0)
    wv = np.zeros((NCORES, epad), np.float32)
    gidx[core_o, pos] = sloc[order]
    sidx[core_o, pos] = (kl[order] * ROWS + dloc[order]).astype(np.int16)
    wv[core_o, pos] = w[order]

    gidx_sb = np.tile(gidx.reshape(NCORES, epad // 16, 16).transpose(0, 2, 1),
                      (1, 8, 1)).copy()
    sidx_sb = np.tile(sidx.reshape(NCORES, epad // 16, 16).transpose(0, 2, 1),
                      (1, 8, 1)).copy()
    w_sb = wv.reshape(NCORES, epad // 128, 128).transpose(0, 2, 1).copy()
    return gidx_sb, sidx_sb, w_sb, K, segchunks, seg_base, epad


def build_program(epad, K, segchunks, seg_base):
    import concourse.mybir as mybir
    import concourse.tile as tile
    import concourse.bacc as bacc

    f32 = mybir.dt.float32
    i16 = mybir.dt.int16
    npair = K // 2
    J = ROWS // 128  # dst rows per partition in zero/reduce layouts
    nc = bacc.Bacc("TRN2", target_bir_lowering=False, debug=False,
                   num_devices=NCORES, num_swdge_queues=1)
    x_ap = nc.dram_tensor("src_x", [N_NODES, D], f32, kind="ExternalInput").ap()
    gidx_ap = nc.dram_tensor("gidx", [128, epad // 16], i16,
                             kind="ExternalInput").ap()
    sidx_ap = nc.dram_tensor("sidx", [128, epad // 16], i16,
                             kind="ExternalInput").ap()
    w_ap = nc.dram_tensor("wv", [128, epad // 128], f32,
                          kind="ExternalInput").ap()
    emb_ap = nc.dram_tensor("emb", [ROWS, D], f32, kind="ExternalOutput").ap()

    with tile.TileContext(nc) as tc:
        with tc.tile_pool(name="dram", bufs=1, space="DRAM") as dpool:
            wide_t = dpool.tile([K * ROWS, D], f32, tag="wide")

            # ---- phase 1: zero wide, gather+scale+scatter ----
            # One big msg tile per quarter: gathers fill slices, one DVE
            # scale covers the quarter, one scatter per plane-pair segment.
            qslots = max(
                (int(seg_base[(qi + 1) * npair]) - int(seg_base[qi * npair]))
                // 128 for qi in range(NQ))
            with (
                tc.tile_pool(name="const", bufs=1) as cpool,
                tc.tile_pool(name="msg", bufs=1) as mpool,
            ):
                zt = cpool.tile([128, 2 * J * D], f32, tag="zt")
                nc.vector.memset(zt[:], 0.0)
                for p2 in range(npair):
                    wslice = wide_t[p2 * 2 * ROWS:(p2 + 1) * 2 * ROWS, :]
                    nc.sync.dma_start(
                        wslice.rearrange("(p j) f -> p (j f)", p=128), zt[:])

                gidx_t = cpool.tile([128, epad // 16], i16, tag="gidx")
                nc.sync.dma_start(gidx_t[:], gidx_ap[:])
                sidx_t = cpool.tile([128, epad // 16], i16, tag="sidx")
                nc.sync.dma_start(sidx_t[:], sidx_ap[:])
                w_t = cpool.tile([128, epad // 128], f32, tag="w")
                nc.sync.dma_start(w_t[:], w_ap[:])

                for qi in range(NQ):
                    qlo = int(seg_base[qi * npair])
                    qhi = int(seg_base[(qi + 1) * npair])
                    nq = qhi - qlo
                    mt = mpool.tile([128, qslots * D], f32, tag="msg")
                    mt3 = mt[:].rearrange("p (s e) -> p s e", e=D)
                    c0 = qlo
                    while c0 < qhi:
                        m = min(CAP, qhi - c0)
                        nc.gpsimd.dma_gather(
                            out_ap=mt3[:, (c0 - qlo) // 128:
                                       (c0 - qlo + m) // 128, :],
                            in_ap=x_ap[qi * SRC_CHUNK:
                                       (qi + 1) * SRC_CHUNK, :],
                            idxs_ap=gidx_t[:, c0 // 16:(c0 + m) // 16],
                            num_idxs=m,
                            num_idxs_reg=m,
                            elem_size=D,
                            single_packet=False,
                            queue_num=0,
                        )
                        c0 += m
                    wb = w_t[:, qlo // 128:qhi // 128]
                    wb3 = wb.unsqueeze(2).broadcast_to([128, nq // 128, D])
                    nc.vector.tensor_tensor(
                        out=mt3[:, :nq // 128, :],
                        in0=mt3[:, :nq // 128, :],
                        in1=wb3,
                        op=mybir.AluOpType.mult,
                    )
                    for p2 in range(npair):
                        si = qi * npair + p2
                        s0 = int(seg_base[si])
                        s1 = int(seg_base[si + 1])
                        if s0 >= s1:
                            continue
                        while s0 < s1:
                            ms = min(CAP, s1 - s0)
                            o = s0 - qlo
                            nc.gpsimd.dma_scatter_add(
                                out_ap=wide_t[p2 * 2 * ROWS:
                                              (p2 + 1) * 2 * ROWS, :],
                                in_ap=mt3[:, o // 128:(o + ms) // 128, :],
                                idxs_ap=sidx_t[:, s0 // 16:(s0 + ms) // 16],
                                num_idxs=ms,
                                num_idxs_reg=ms,
                                elem_size=D,
                                single_packet=False,
                                queue_num=0,
                            )
                            s0 += ms

            # ---- phase 2: reduce K planes -> emb ----
            with tc.tile_pool(name="red", bufs=1) as rpool:
                KB = 6  # planes per load block
                acc = rpool.tile([128, J * D], f32, tag="acc")
                part = rpool.tile([128, J * D], f32, tag="part")
                src = wide_t[:].rearrange("(k p j) f -> p k (j f)", k=K,
                                          p=128)
                nblocks = -(-K // KB)
                for b in range(nblocks):
                    kb = min(KB, K - b * KB)
                    rt = rpool.tile([128, KB * J * D], f32, tag="rt")
                    rt4 = rt[:, :kb * J * D].rearrange(
                        "p (k j f) -> p k (j f)", k=kb, f=D)
                    nc.sync.dma_start(rt4[:, :, :],
                                      src[:, b * KB:min((b + 1) * KB, K), :])
                    red_in = rt[:, :kb * J * D].rearrange("p (k jf) -> p jf k", k=kb)
                    out_t = acc if b == 0 else part
                    nc.vector.tensor_reduce(
                        out=out_t[:], in_=red_in,
                        axis=mybir.AxisListType.X,
                        op=mybir.AluOpType.add)
                    if b > 0:
                        nc.vector.tensor_tensor(out=acc[:], in0=acc[:],
                                                in1=part[:],
                                                op=mybir.AluOpType.add)
                nc.sync.dma_start(
                    emb_ap[:].rearrange("(p j) f -> p (j f)", p=128), acc[:])

    nc.compile()
    return nc


def kernel(src_x, dst_x, edge_index, edge_weight):
    from concourse.bass_utils import run_bass_kernel_spmd

    src_x = np.ascontiguousarray(np.asarray(src_x, dtype=np.float32))
    gidx_sb, sidx_sb, w_sb, K, segchunks, seg_base, epad = _prep(
        edge_index, edge_weight)
    nc = build_program(epad, K, segchunks, seg_base)
    in_maps = [
        {"src_x": src_x, "gidx": gidx_sb[c], "sidx": sidx_sb[c], "wv": w_sb[c]}
        for c in range(NCORES)
    ]
    res = run_bass_kernel_spmd(nc, in_maps, core_ids=list(range(NCORES)))
    out = np.concatenate([res.results[c]["emb"][:NPC] for c in range(NCORES)],
                         axis=0)
    return out
